# revision 1
# baseline (speedup 1.0000x reference)
"""TRN2 Bass kernel for nn_Decoder_83279415870148.

6-layer causal transformer decoder (B=8, S=1024, D=512, H=8, DFF=2048).
Sharding: pure data-parallel — one batch element per NeuronCore, weights
replicated, no collectives.

Per-core layout: activations transposed [feature -> partitions, tokens -> free].
- matmuls: weight-stationary, lhsT = weight chunks [128,128] bf16, rhs =
  activations [128, 512] bf16, fp32 PSUM accumulation
- attention: scores computed transposed [k_tok, q_tok]; exp fused into PSUM
  eviction (scale=1/8, no max subtraction — scores are bounded ~1.6); causal
  mask via -1e30 triangle accumulated on the diagonal block through the PE;
  softmax denominator via a ones-column appended to V (M=65 matmul);
  normalization in fp32 (DVE reciprocal + gpsimd partition_broadcast)
- LayerNorm: cross-partition sums via all-ones lhsT matmul (sum broadcast to
  all partitions in one shot), stats fp32, single-rounding apply
- residual stream kept in fp32 tiles; bf16 copies feed matmuls
- q-bias kept (per-partition, fused into eviction); k-bias dropped (cancels
  in softmax); v-bias folded into out-proj bias host-side
"""
import numpy as np
import ml_dtypes
from contextlib import ExitStack

import concourse.bass as bass
import concourse.tile as tile
from concourse import bacc, mybir
from concourse.bass_utils import run_bass_kernel_spmd

P = 128
B, S, D, H, L = 8, 1024, 512, 8, 6
DK = D // H          # 64
DFF = 4 * D          # 2048
DC = D // P          # 4 feature chunks
FC = DFF // P        # 16 dff chunks
NT = S // P          # 8 token chunks
TH = 512             # token half (matmul stream length)
EPS = 1e-5
BF = mybir.dt.bfloat16
F32 = mybir.dt.float32
AF = mybir.ActivationFunctionType


def build(nlayers=L, debug_outputs=()):
    nc = bacc.Bacc("TRN2", target_bir_lowering=False, debug=False, num_devices=8)
    dt = nc.dram_tensor
    xT_d = dt("xT", [D, S], BF, kind="ExternalInput").ap()
    xT32_d = dt("xT32", [D, S], F32, kind="ExternalInput").ap()
    wqk_d = dt("wqk", [L, D, 2 * D], BF, kind="ExternalInput").ap()
    wv_d = dt("wv", [L, D, D], BF, kind="ExternalInput").ap()
    wo_d = dt("wo", [L, D, D], BF, kind="ExternalInput").ap()
    w1_d = dt("w1", [L, D, DFF], BF, kind="ExternalInput").ap()
    w2_d = dt("w2", [L, DFF, D], BF, kind="ExternalInput").ap()
    bq_d = dt("bq", [L, D], F32, kind="ExternalInput").ap()
    bo_d = dt("bo", [L, D], BF, kind="ExternalInput").ap()
    b1_d = dt("b1", [L, DFF], F32, kind="ExternalInput").ap()
    b2_d = dt("b2", [L, D], BF, kind="ExternalInput").ap()
    ln_d = dt("lnp", [L, 4, D], F32, kind="ExternalInput").ap()
    ones_d = dt("ones", [P, TH], BF, kind="ExternalInput").ap()
    ident_d = dt("ident", [P, P], BF, kind="ExternalInput").ap()
    tri_d = dt("tri", [P, P], BF, kind="ExternalInput").ap()
    out_d = dt("outT", [D, S], F32, kind="ExternalOutput").ap()
    dbg_d = {nm: dt(f"dbg_{nm}", [P * nchunks, S], F32, kind="ExternalOutput").ap()
             for nm, nchunks in debug_outputs}

    with tile.TileContext(nc) as tc, ExitStack() as ctx:
        cp = ctx.enter_context(tc.tile_pool(name="cp", bufs=1))      # persistent
        wp2 = ctx.enter_context(tc.tile_pool(name="wp2", bufs=1))    # dbl-buf weights
        wp1 = ctx.enter_context(tc.tile_pool(name="wp1", bufs=1))    # big weights
        ap = ctx.enter_context(tc.tile_pool(name="ap", bufs=1))      # work tiles
        psA = ctx.enter_context(tc.tile_pool(name="psA", bufs=8, space="PSUM"))

        # consts
        ones = cp.tile([P, TH], BF, name="ones")
        ident = cp.tile([P, P], BF, name="ident")
        tri = cp.tile([P, P], BF, name="tri")
        eps_t = cp.tile([P, 1], F32, name="eps_t")
        nc.vector.memset(eps_t[:], EPS)
        nc.sync.dma_start(ones[:], ones_d)
        nc.sync.dma_start(ident[:], ident_d)
        nc.sync.dma_start(tri[:], tri_d)

        # persistent activation tiles
        xbf = [cp.tile([P, S], BF, name=f"xbf{c}") for c in range(DC)]
        x32 = [cp.tile([P, S], F32, name=f"x32{c}") for c in range(DC)]
        x1bf = [cp.tile([P, S], BF, name=f"x1bf{c}") for c in range(DC)]
        x132 = [cp.tile([P, S], F32, name=f"x132{c}") for c in range(DC)]
        qT = [cp.tile([P, S], BF, name=f"qT{c}") for c in range(DC)]
        kT = [cp.tile([P, S], BF, name=f"kT{c}") for c in range(DC)]
        oT = [cp.tile([P, S], BF, name=f"oT{c}") for c in range(DC)]
        v65 = [cp.tile([P, H * 65], BF, name=f"v65_{t}") for t in range(NT)]

        for t in range(NT):
            v3 = v65[t][:].rearrange("p (h c) -> p h c", h=H)
            nc.vector.memset(v3[:, :, 64:65], 1.0)

        for c in range(DC):
            nc.sync.dma_start(xbf[c][:], xT_d[c * P:(c + 1) * P])
            nc.sync.dma_start(x32[c][:], xT32_d[c * P:(c + 1) * P])

        def dbg_dump(nm, tiles):
            if nm in dbg_d:
                for c, t in enumerate(tiles):
                    tf = ap.tile([P, S], F32, name=f"dbg{nm}{c}", tag="dbgf")
                    nc.vector.tensor_copy(tf[:], t[:])
                    nc.sync.dma_start(dbg_d[nm][c * P:(c + 1) * P], tf[:])

        for i in range(nlayers):
            last = (i == nlayers - 1)
            # ---------------- per-layer weights/biases ----------------
            wqk = [wp2.tile([P, 2 * D], BF, name=f"wqk{kc}", tag=f"wqk{kc}", bufs=2)
                   for kc in range(DC)]
            wv = [wp2.tile([P, D], BF, name=f"wv{kc}", tag=f"wv{kc}", bufs=2)
                  for kc in range(DC)]
            wo = [wp2.tile([P, D], BF, name=f"wo{kc}", tag=f"wo{kc}")
                  for kc in range(DC)]
            w1 = [wp1.tile([P, DFF], BF, name=f"w1_{kc}", tag=f"w1_{kc}")
                  for kc in range(DC)]
            w2 = [wp1.tile([P, D], BF, name=f"w2_{fc}", tag=f"w2_{fc}")
                  for fc in range(FC)]
            bq_t = wp2.tile([P, DC], F32, name="bq_t", tag="bq_t")
            bo_r = wp2.tile([1, D], BF, name="bo_r", tag="bo_r")
            b1_t = wp2.tile([P, FC], F32, name="b1_t", tag="b1_t")
            b2_r = wp2.tile([1, D], BF, name="b2_r", tag="b2_r")
            ln_t = wp2.tile([P, 4 * DC], F32, name="ln_t", tag="ln_t")

            for kc in range(DC):
                nc.sync.dma_start(wqk[kc][:], wqk_d[i % L, kc * P:(kc + 1) * P])
                nc.sync.dma_start(wv[kc][:], wv_d[i % L, kc * P:(kc + 1) * P])
                nc.sync.dma_start(wo[kc][:], wo_d[i % L, kc * P:(kc + 1) * P])
                nc.sync.dma_start(w1[kc][:], w1_d[i % L, kc * P:(kc + 1) * P])
            for fc in range(FC):
                nc.sync.dma_start(w2[fc][:], w2_d[i % L, fc * P:(fc + 1) * P])
            nc.sync.dma_start(bq_t[:], bq_d[i % L].rearrange("(c p) -> p c", p=P))
            nc.sync.dma_start(bo_r[:], bo_d[i % L].rearrange("(o m) -> o m", o=1))
            nc.sync.dma_start(b1_t[:], b1_d[i % L].rearrange("(c p) -> p c", p=P))
            nc.sync.dma_start(b2_r[:], b2_d[i % L].rearrange("(o m) -> o m", o=1))
            nc.sync.dma_start(ln_t[:], ln_d[i % L].rearrange("g (c p) -> p (g c)", p=P))

            # ---------------- phase A: Q.T, K.T ----------------
            for hf in range(2):
                sl = slice(hf * TH, (hf + 1) * TH)
                for mc in range(2 * DC):
                    pt = psA.tile([P, TH], F32, name="pA", tag="mm")
                    for kc in range(DC):
                        nc.tensor.matmul(pt[:], wqk[kc][:, mc * P:(mc + 1) * P],
                                         xbf[kc][:, sl],
                                         start=(kc == 0), stop=(kc == DC - 1))
                    if mc < DC:
                        nc.vector.tensor_scalar_add(qT[mc][:, sl], pt[:],
                                                    bq_t[:, mc:mc + 1])
                    else:
                        nc.vector.tensor_copy(kT[mc - DC][:, sl], pt[:])

            # ---------------- phase B: V (token layout + ones col) ----------------
            for t in range(NT):
                pv = psA.tile([P, D], F32, name="pV", tag="mm")
                for kc in range(DC):
                    nc.tensor.matmul(pv[:], xbf[kc][:, t * P:(t + 1) * P], wv[kc][:],
                                     start=(kc == 0), stop=(kc == DC - 1))
                v3 = v65[t][:].rearrange("p (h c) -> p h c", h=H)
                nc.vector.tensor_copy(v3[:, :, 0:64],
                                      pv[:].rearrange("p (h c) -> p h c", h=H))

            # ---------------- phase C: attention per head ----------------
            for hp in range(H // 2):
                ti = hp
                # two heads (rows 0-63 / 64-127 of tile ti) interleaved so the
                # K=64 score matmuls pack into disjoint PE row groups
                pts2 = {0: [], 1: []}
                for j in range(NT):
                    q0 = j * P
                    rem = S - q0
                    s1 = min(TH, rem)
                    spans = [(q0, s1)]
                    if rem > s1:
                        spans.append((q0 + s1, rem - s1))
                    ptiles = {}
                    for sub in range(2):
                        # tile j only holds the causal q-range [q0, S)
                        ptiles[sub] = ap.tile([P, rem], BF, name=f"pt{sub}_{j}",
                                              tag=f"pt{sub}_{j}")
                        pts2[sub].append(ptiles[sub])
                    for (qs, sl_len) in spans:
                        pps = {}
                        for sub in range(2):
                            ko = 64 * sub
                            pp = psA.tile([P, sl_len], F32, name="pS",
                                          tag="mm")
                            nc.tensor.matmul(pp[:],
                                             kT[ti][ko:ko + 64, q0:q0 + P],
                                             qT[ti][ko:ko + 64, qs:qs + sl_len],
                                             start=True, stop=(qs != q0))
                            pps[sub] = pp
                        for sub in range(2):
                            pp = pps[sub]
                            if qs == q0:
                                nc.tensor.matmul(pp[:, 0:P], ident[:], tri[:],
                                                 start=False, stop=True,
                                                 skip_group_check=True)
                            nc.scalar.activation(
                                ptiles[sub][:, qs - q0:qs - q0 + sl_len],
                                pp[:], AF.Exp, scale=0.125)
                for sub in range(2):
                    h = 2 * hp + sub
                    ko = 64 * sub
                    pts = pts2[sub]
                    for hf in range(2):
                        jhi = 4 * hf + 3
                        po = psA.tile([65, TH], F32, name="pO", tag="mm")
                        for j in range(jhi + 1):
                            rs = max(hf * TH, j * P)
                            nc.tensor.matmul(
                                po[:, rs - hf * TH:TH],
                                v65[j][:, 65 * h:65 * h + 65],
                                pts[j][:, rs - j * P:(hf + 1) * TH - j * P],
                                start=(j == 0), stop=(j == jhi),
                                skip_group_check=(j > 0))
                        rrow = ap.tile([1, TH], F32, name="rrow", tag="rrow")
                        nc.vector.reciprocal(rrow[:], po[64:65, :])
                        rb = ap.tile([64, TH], F32, name="rb", tag="rb")
                        nc.gpsimd.partition_broadcast(rb[:], rrow[:])
                        nc.vector.tensor_mul(
                            oT[ti][ko:ko + 64, hf * TH:(hf + 1) * TH],
                            po[0:64, :], rb[:])

            if "qT" in dbg_d:
                dbg_dump("qT", qT)
                dbg_dump("kT", kT)
                dbg_dump("oT", oT)

            # ---------------- phase D: out-proj + resid + LN1 ----------------
            def ln_half(hf, ysrc, g_off, b_off, dst32, dstbf):
                sl = slice(hf * TH, (hf + 1) * TH)
                ybf, ysq = [], []
                for mc in range(DC):
                    yb = ap.tile([P, TH], BF, name=f"yb{mc}", tag=f"h{mc}")
                    nc.scalar.copy(yb[:], ysrc[mc][:])
                    ybf.append(yb)
                    sq = ap.tile([P, TH], BF, name=f"sq{mc}", tag=f"h{4 + mc}")
                    nc.scalar.activation(sq[:], ysrc[mc][:], AF.Square)
                    ysq.append(sq)
                ps_s = psA.tile([P, TH], F32, name="lnS", tag="mm")
                for mc in range(DC):
                    nc.tensor.matmul(ps_s[:], ones[:, 0:P], ybf[mc][:],
                                     start=(mc == 0), stop=(mc == DC - 1))
                ps_q = psA.tile([P, TH], F32, name="lnQ", tag="mm")
                for mc in range(DC):
                    nc.tensor.matmul(ps_q[:], ones[:, 0:P], ysq[mc][:],
                                     start=(mc == 0), stop=(mc == DC - 1))
                stA = ap.tile([P, TH], F32, name="stA", tag="stA")
                stB = ap.tile([P, TH], F32, name="stB", tag="stB")
                stC = ap.tile([P, TH], F32, name="stC", tag="stC")
                stD = ap.tile([P, TH], F32, name="stD", tag="stD")
                nc.vector.tensor_scalar_mul(stA[:], ps_s[:], 1.0 / D)
                nc.vector.tensor_scalar_mul(stB[:], ps_q[:], 1.0 / D)
                nc.vector.tensor_mul(stC[:], stA[:], stA[:])
                nc.vector.tensor_sub(stD[:], stB[:], stC[:])
                nc.scalar.activation(stB[:], stD[:], AF.Sqrt, bias=eps_t[:])
                nc.vector.reciprocal(stC[:], stB[:])
                for mc in range(DC):
                    # DVE computes the (y-m)*rstd chain; the g/b affines run
                    # on ACT (idle during LN) so they pipeline behind it
                    lt = ap.tile([P, TH], F32, name="lt", tag="lt", bufs=2)
                    nc.vector.tensor_sub(lt[:], ysrc[mc][:], stA[:])
                    lu = ap.tile([P, TH], F32, name="lu", tag="lu", bufs=2)
                    nc.vector.tensor_mul(lu[:], lt[:], stC[:])
                    gcol = ln_t[:, g_off * DC + mc:g_off * DC + mc + 1]
                    bcol = ln_t[:, b_off * DC + mc:b_off * DC + mc + 1]
                    if dstbf is not None:
                        nc.scalar.activation(dstbf[mc][:, sl], lu[:],
                                             AF.Identity, bias=bcol, scale=gcol)
                    nc.scalar.activation(dst32[mc][:, sl], lu[:],
                                         AF.Identity, bias=bcol, scale=gcol)

            def proj_resid_ln(w_lhsT, nk, b_row, rhs_tiles, res32, dst32, dstbf,
                              g_off, b_off, final=False):
                # matmul proj + bias; y32 = proj + res32; LN -> dst
                for hf in range(2):
                    sl = slice(hf * TH, (hf + 1) * TH)
                    y32h = []
                    for mc in range(DC):
                        pt = psA.tile([P, TH], F32, name="pP", tag="mm")
                        for kc in range(nk):
                            nc.tensor.matmul(pt[:],
                                             w_lhsT[kc][:, mc * P:(mc + 1) * P],
                                             rhs_tiles[kc][:, sl],
                                             start=(kc == 0), stop=False)
                        nc.tensor.matmul(pt[:], b_row[0:1, mc * P:(mc + 1) * P],
                                         ones[0:1, :], start=False, stop=True,
                                         skip_group_check=True)
                        yt = ap.tile([P, TH], F32, name=f"y32_{mc}",
                                     tag=f"y32_{mc}")
                        nc.vector.tensor_add(yt[:], pt[:], res32[mc][:, sl])
                        y32h.append(yt)
                    ln_half(hf, y32h, g_off, b_off, dst32,
                            None if final else dstbf)

            proj_resid_ln(wo, DC, bo_r, oT, x32, x132, x1bf, 0, 1)

            if "x1" in dbg_d:
                dbg_dump("x1", x132)

            # ---------------- phase E: MLP ----------------
            # h chunks then accumulate into y2; resid x1; LN2 -> next x
            hts_all = {}
            for hf in range(2):
                sl = slice(hf * TH, (hf + 1) * TH)
                hts = []
                for fc in range(FC):
                    ph = psA.tile([P, TH], F32, name="pH", tag="mm")
                    for kc in range(DC):
                        nc.tensor.matmul(ph[:], w1[kc][:, fc * P:(fc + 1) * P],
                                         x1bf[kc][:, sl],
                                         start=(kc == 0), stop=(kc == DC - 1))
                    ht = ap.tile([P, TH], BF, name=f"ht{fc}", tag=f"h{fc}")
                    nc.scalar.activation(ht[:], ph[:], AF.Relu,
                                         bias=b1_t[:, fc:fc + 1])
                    hts.append(ht)
                hts_all[hf] = hts

                # y2 accumulation for this half + resid + LN2 (dst = next x)
                for mc in range(DC):
                    pt = psA.tile([P, TH], F32, name="pY", tag="mm")
                    for fc in range(FC):
                        nc.tensor.matmul(pt[:], w2[fc][:, mc * P:(mc + 1) * P],
                                         hts[fc][:], start=(fc == 0), stop=False)
                    nc.tensor.matmul(pt[:], b2_r[0:1, mc * P:(mc + 1) * P],
                                     ones[0:1, :], start=False, stop=True,
                                     skip_group_check=True)
                    yt = ap.tile([P, TH], F32, name=f"y32b_{mc}",
                                 tag=f"y32_{mc}")
                    nc.vector.tensor_add(yt[:], pt[:], x132[mc][:, sl])
                    # stash for LN below via tag reuse: recompute handle
                    hts_all[(hf, mc)] = yt

                # LN2 for this half
                y2h = [hts_all[(hf, mc)] for mc in range(DC)]
                ln_half(hf, y2h, 2, 3, x32, None if last else xbf)

        for c in range(DC):
            nc.sync.dma_start(out_d[c * P:(c + 1) * P], x32[c][:])

    nc.compile()
    return nc


_CACHE = {}


def _host_prep(qkv_w, qkv_b, out_w, out_b, ln1_g, ln1_b, mlp_w1, mlp_b1,
               mlp_w2, mlp_b2, ln2_g, ln2_b):
    bfc = lambda a: np.ascontiguousarray(np.asarray(a)).astype(ml_dtypes.bfloat16)
    f32c = lambda a: np.ascontiguousarray(np.asarray(a), dtype=np.float32)
    qkv_w = np.asarray(qkv_w)
    bo_eff = np.asarray(out_b) + np.einsum(
        "ld,ldo->lo", np.asarray(qkv_b)[:, 2 * D:].astype(np.float64),
        np.asarray(out_w).astype(np.float64)).astype(np.float32)
    common = {
        "wqk": bfc(qkv_w[:, :, :2 * D]),
        "wv": bfc(qkv_w[:, :, 2 * D:]),
        "wo": bfc(out_w),
        "w1": bfc(mlp_w1),
        "w2": bfc(mlp_w2),
        "bq": f32c(np.asarray(qkv_b)[:, :D]),
        "bo": bfc(bo_eff),
        "b1": f32c(mlp_b1),
        "b2": bfc(mlp_b2),
        "lnp": f32c(np.stack([ln1_g, ln1_b, ln2_g, ln2_b], axis=1)),
        "ones": np.ones((P, TH), ml_dtypes.bfloat16),
        "ident": np.eye(P).astype(ml_dtypes.bfloat16),
        "tri": (np.tril(np.ones((P, P), np.float32), -1) * -1e30
                ).astype(ml_dtypes.bfloat16),
    }
    return common


def kernel(x, qkv_w, qkv_b, out_w, out_b, ln1_g, ln1_b, mlp_w1, mlp_b1,
           mlp_w2, mlp_b2, ln2_g, ln2_b):
    if "nc" not in _CACHE:
        _CACHE["nc"] = build()
    nc = _CACHE["nc"]
    common = _host_prep(qkv_w, qkv_b, out_w, out_b, ln1_g, ln1_b, mlp_w1,
                        mlp_b1, mlp_w2, mlp_b2, ln2_g, ln2_b)
    x = np.asarray(x)
    in_maps = []
    for c in range(B):
        xt = np.ascontiguousarray(x[c].T)
        in_maps.append({**common,
                        "xT": xt.astype(ml_dtypes.bfloat16),
                        "xT32": xt.astype(np.float32)})
    res = run_bass_kernel_spmd(nc, in_maps, core_ids=list(range(B)))
    out = np.stack([np.ascontiguousarray(r["outT"].T) for r in res.results])
    return out.astype(np.float32)



# revision 5
# speedup vs baseline: 1.0035x; 1.0035x over previous
"""TRN2 Bass kernel for nn_Decoder_83279415870148.

6-layer causal transformer decoder (B=8, S=1024, D=512, H=8, DFF=2048).
Sharding: pure data-parallel - one batch element per NeuronCore, weights
replicated, no collectives.

v2: fp8(e4m3) DoubleRow matmuls for QKV / V / out-proj / A@V (numerics
validated against the reference dataflow in fp32 simulation: ~7e-3 rel),
MLP + scores kept bf16, softmax weights fp8 with scale 32 folded into the
exp bias, attention software-pipelined (scores of head-pair hp overlap
A@V of hp-1 via double-buffered probability tiles).
Layout: activations transposed [feature -> partitions, tokens -> free].
"""
import numpy as np
import ml_dtypes
from contextlib import ExitStack

import concourse.bass as bass
import concourse.tile as tile
from concourse import bacc, mybir
from concourse.bass_utils import run_bass_kernel_spmd

P = 128
B, S, D, H, L = 8, 1024, 512, 8, 6
DK = D // H          # 64
DFF = 4 * D          # 2048
DC = D // P          # 4 feature chunks
FC = DFF // P        # 16 dff chunks
NT = S // P          # 8 token chunks
TH = 512             # token half (matmul stream length)
EPS = 1e-5
BF = mybir.dt.bfloat16
F8 = mybir.dt.float8e4
F32 = mybir.dt.float32
AF = mybir.ActivationFunctionType
DR = mybir.MatmulPerfMode.DoubleRow
OP = mybir.AluOpType

S_W = 512.0          # fp8 weight scale (qkv/v/o)
S_X = 16.0           # fp8 activation scale (x into qkv)
S_V = 32.0           # fp8 v scale
S_ATT = 32.0         # fp8 softmax-weight scale (via exp bias)
S_O = 32.0           # fp8 attention-out scale
DS_QK = 1.0 / (S_X * S_W)
SV_DS = S_V / (S_X * S_W)
DS_O = 1.0 / (S_O * S_W)
LN_SATT = float(np.log(S_ATT))
VW = H * 66          # v pair-tile width per half (64 v + 1 ones + 1 pad)


def build(nlayers=L):
    nc = bacc.Bacc("TRN2", target_bir_lowering=False, debug=False, num_devices=8)
    dt = nc.dram_tensor
    x8_d = dt("x8", [2, P, 2 * S], F8, kind="ExternalInput").ap()
    xT32_d = dt("xT32", [D, S], F32, kind="ExternalInput").ap()
    wqk8_d = dt("wqk8", [L, 2, P, 2 * 2 * D], F8, kind="ExternalInput").ap()
    wv8_d = dt("wv8", [L, 2, P, 2 * D], F8, kind="ExternalInput").ap()
    wo8_d = dt("wo8", [L, 2, P, 2 * D], F8, kind="ExternalInput").ap()
    w1_d = dt("w1", [L, D, DFF], BF, kind="ExternalInput").ap()
    w2_d = dt("w2", [L, DFF, D], BF, kind="ExternalInput").ap()
    bq_d = dt("bq", [L, D], F32, kind="ExternalInput").ap()
    bo_d = dt("bo", [L, D], BF, kind="ExternalInput").ap()
    b1_d = dt("b1", [L, DFF], F32, kind="ExternalInput").ap()
    b2_d = dt("b2", [L, D], BF, kind="ExternalInput").ap()
    ln_d = dt("lnp", [L, 6, D], F32, kind="ExternalInput").ap()
    ones_d = dt("ones", [P, TH], BF, kind="ExternalInput").ap()
    ident_d = dt("ident", [P, P], BF, kind="ExternalInput").ap()
    tri_d = dt("tri", [P, P], BF, kind="ExternalInput").ap()
    out_d = dt("outT", [D, S], F32, kind="ExternalOutput").ap()

    with tile.TileContext(nc) as tc, ExitStack() as ctx:
        cp = ctx.enter_context(tc.tile_pool(name="cp", bufs=1))      # persistent
        wp2 = ctx.enter_context(tc.tile_pool(name="wp2", bufs=1))    # dbl-buf weights
        wp1 = ctx.enter_context(tc.tile_pool(name="wp1", bufs=1))    # big weights
        ap = ctx.enter_context(tc.tile_pool(name="ap", bufs=1))      # work tiles
        psA = ctx.enter_context(tc.tile_pool(name="psA", bufs=4, space="PSUM"))
        psB = ctx.enter_context(tc.tile_pool(name="psB", bufs=4, space="PSUM"))

        # consts
        ones = cp.tile([P, TH], BF, name="ones")
        ident = cp.tile([P, P], BF, name="ident")
        tri = cp.tile([P, P], BF, name="tri")
        eps_t = cp.tile([P, 1], F32, name="eps_t")
        nc.vector.memset(eps_t[:], EPS)
        lnsc_t = cp.tile([P, 1], F32, name="lnsc_t")
        nc.vector.memset(lnsc_t[:], LN_SATT)
        nc.sync.dma_start(ones[:], ones_d)
        nc.sync.dma_start(ident[:], ident_d)
        nc.sync.dma_start(tri[:], tri_d)

        # persistent activation tiles
        x8 = [cp.tile([P, 2 * S], F8, name=f"x8_{u}") for u in range(2)]
        x32 = [cp.tile([P, S], F32, name=f"x32{c}") for c in range(DC)]
        x1bf = [cp.tile([P, S], BF, name=f"x1bf{c}") for c in range(DC)]
        x132 = [cp.tile([P, S], F32, name=f"x132{c}") for c in range(DC)]
        qT = [cp.tile([P, S], BF, name=f"qT{c}") for c in range(DC)]
        kT = [cp.tile([P, S], BF, name=f"kT{c}") for c in range(DC)]
        oT8 = [cp.tile([P, 2 * S], F8, name=f"oT8_{u}") for u in range(2)]
        vp = [cp.tile([P, 2 * VW], F8, name=f"vp{u}") for u in range(4)]
        # attention probability pair-tiles, double-buffered across head-pairs
        SPANS = [S, S - 256, S - 512, S - 768]
        pt8 = {}
        for bb in range(2):
            for sub in range(2):
                for u in range(4):
                    t = cp.tile([P, 2 * SPANS[u]], F8, name=f"pt{bb}{sub}{u}")
                    nc.vector.memset(t[:, SPANS[u]:SPANS[u] + P], 0.0)
                    pt8[(bb, sub, u)] = t
        for u in range(4):
            nc.vector.memset(vp[u][:], 0.0)
            v3 = vp[u][:].rearrange("p (two h c) -> p two h c", two=2, h=H)
            nc.vector.memset(v3[:, :, :, 64:65], 1.0)

        for u in range(2):
            nc.sync.dma_start(x8[u][:], x8_d[u])
        for c in range(DC):
            nc.sync.dma_start(x32[c][:], xT32_d[c * P:(c + 1) * P])

        for i in range(nlayers):
            last = (i == nlayers - 1)
            # ---------------- per-layer weights/biases ----------------
            wqk8 = [wp2.tile([P, 2 * 2 * D], F8, name=f"wqk8{u}", tag=f"wqk{u}",
                             bufs=2) for u in range(2)]
            wv8 = [wp2.tile([P, 2 * D], F8, name=f"wv8{u}", tag=f"wv{u}", bufs=2)
                   for u in range(2)]
            wo8 = [wp2.tile([P, 2 * D], F8, name=f"wo8{u}", tag=f"wo{u}")
                   for u in range(2)]
            w1 = [wp1.tile([P, DFF], BF, name=f"w1_{kc}", tag=f"w1_{kc}")
                  for kc in range(DC)]
            w2 = [wp1.tile([P, D], BF, name=f"w2_{fc}", tag=f"w2_{fc}")
                  for fc in range(FC)]
            bq_t = wp2.tile([P, DC], F32, name="bq_t", tag="bq_t")
            bo_r = wp2.tile([1, D], BF, name="bo_r", tag="bo_r")
            b1_t = wp2.tile([P, FC], F32, name="b1_t", tag="b1_t")
            b2_r = wp2.tile([1, D], BF, name="b2_r", tag="b2_r")
            ln_t = wp2.tile([P, 6 * DC], F32, name="ln_t", tag="ln_t")

            for u in range(2):
                nc.sync.dma_start(wqk8[u][:], wqk8_d[i % L, u])
                nc.sync.dma_start(wv8[u][:], wv8_d[i % L, u])
                nc.sync.dma_start(wo8[u][:], wo8_d[i % L, u])
            for kc in range(DC):
                nc.sync.dma_start(w1[kc][:], w1_d[i % L, kc * P:(kc + 1) * P])
            for fc in range(FC):
                nc.sync.dma_start(w2[fc][:], w2_d[i % L, fc * P:(fc + 1) * P])
            nc.sync.dma_start(bq_t[:], bq_d[i % L].rearrange("(c p) -> p c", p=P))
            nc.sync.dma_start(bo_r[:], bo_d[i % L].rearrange("(o m) -> o m", o=1))
            nc.sync.dma_start(b1_t[:], b1_d[i % L].rearrange("(c p) -> p c", p=P))
            nc.sync.dma_start(b2_r[:], b2_d[i % L].rearrange("(o m) -> o m", o=1))
            nc.sync.dma_start(ln_t[:], ln_d[i % L].rearrange("g (c p) -> p (g c)", p=P))

            w3qk = [wqk8[u][:].rearrange("p (two m) -> p two m", two=2)
                    for u in range(2)]
            w3v = [wv8[u][:].rearrange("p (two m) -> p two m", two=2)
                   for u in range(2)]
            w3o = [wo8[u][:].rearrange("p (two m) -> p two m", two=2)
                   for u in range(2)]
            x83 = [x8[u][:].rearrange("p (two m) -> p two m", two=2)
                   for u in range(2)]

            # ---------------- phase A: Q.T, K.T (fp8 DoubleRow) ----------------
            for hf in range(2):
                sl = slice(hf * TH, (hf + 1) * TH)
                for mc in range(2 * DC):
                    pt = psA.tile([P, TH], F32, name="pA", tag="mm")
                    for u in range(2):
                        nc.tensor.matmul(pt[:], w3qk[u][:, :, mc * P:(mc + 1) * P],
                                         x83[u][:, :, sl], start=(u == 0),
                                         stop=(u == 1), perf_mode=DR)
                    if mc < DC:
                        nc.vector.tensor_scalar(qT[mc][:, sl], pt[:], DS_QK,
                                                bq_t[:, mc:mc + 1], OP.mult, OP.add)
                    else:
                        nc.vector.tensor_scalar_mul(kT[mc - DC][:, sl], pt[:], DS_QK)

            # ---------------- phase B: V (token layout, fp8 DR) ----------------
            for t in range(NT):
                pv = psA.tile([P, D], F32, name="pV", tag="mm")
                for u in range(2):
                    nc.tensor.matmul(pv[:], x83[u][:, :, t * P:(t + 1) * P],
                                     w3v[u][:], start=(u == 0), stop=(u == 1),
                                     perf_mode=DR)
                v3 = vp[t // 2][:].rearrange("p (two h c) -> p two h c",
                                             two=2, h=H)
                nc.vector.tensor_scalar_mul(
                    v3[:, t % 2, :, 0:64],
                    pv[:].rearrange("p (h c) -> p h c", h=H), SV_DS)

            # ---------------- phase C: attention (pipelined over hp) ----------
            def emit_scores(hp):
                bb = hp % 2
                ti = hp
                for j in range(NT):
                    u, half = j // 2, j % 2
                    q0 = j * P
                    rem = S - q0
                    s1 = min(TH, rem)
                    spans = [(q0, s1)]
                    if rem > s1:
                        spans.append((q0 + s1, rem - s1))
                    p3 = {sub: pt8[(bb, sub, u)][:].rearrange(
                        "p (two m) -> p two m", two=2) for sub in range(2)}
                    for (qs, sl_len) in spans:
                        pps = {}
                        for sub in range(2):
                            ko = 64 * sub
                            pp = psA.tile([P, sl_len], F32, name="pS", tag="mm")
                            nc.tensor.matmul(pp[:],
                                             kT[ti][ko:ko + 64, q0:q0 + P],
                                             qT[ti][ko:ko + 64, qs:qs + sl_len],
                                             start=True, stop=(qs != q0))
                            pps[sub] = pp
                        for sub in range(2):
                            pp = pps[sub]
                            if qs == q0:
                                nc.tensor.matmul(pp[:, 0:P], ident[:], tri[:],
                                                 start=False, stop=True,
                                                 skip_group_check=True)
                            co = qs - 256 * u
                            nc.scalar.activation(
                                p3[sub][:, half, co:co + sl_len],
                                pp[:], AF.Exp, scale=0.125, bias=lnsc_t[:])

            def emit_av(hp):
                bb = hp % 2
                for sub in range(2):
                    h = 2 * hp + sub
                    u2, hh, ro = h // 4, (h // 2) % 2, 64 * (h % 2)
                    for hf in range(2):
                        nu = 2 if hf == 0 else 4
                        po = psB.tile([66, TH], F32, name="pO", tag="po")
                        for u in range(nu):
                            rs = max(hf * TH, u * 256)
                            pt3 = pt8[(bb, sub, u)][:].rearrange(
                                "p (two m) -> p two m", two=2)
                            v3 = vp[u][:].rearrange(
                                "p (two m) -> p two m", two=2)
                            nc.tensor.matmul(
                                po[:, rs - hf * TH:TH],
                                v3[:, :, h * 66:h * 66 + 66],
                                pt3[:, :, rs - 256 * u:(hf + 1) * TH - 256 * u],
                                start=(u == 0), stop=(u == nu - 1),
                                perf_mode=DR, skip_group_check=(u > 0))
                        rrow = ap.tile([1, TH], F32, name="rrow", tag="rrow")
                        nc.vector.reciprocal(rrow[:], po[64:65, :])
                        rb = ap.tile([64, TH], F32, name="rb", tag="rb")
                        nc.gpsimd.partition_broadcast(rb[:], rrow[:])
                        nc.vector.scalar_tensor_tensor(
                            oT8[u2][ro:ro + 64,
                                    hh * S + hf * TH:hh * S + (hf + 1) * TH],
                            po[0:64, :], S_O / S_V, rb[:], OP.mult, OP.mult)

            emit_scores(0)
            for hp in range(1, H // 2):
                emit_scores(hp)
                emit_av(hp - 1)
            emit_av(H // 2 - 1)

            # ---------------- phase D: out-proj (fp8 DR) + resid + LN1 --------
            def ln_half(hf, ysrc, g_off, b_off, dst32, dstbf, dst8=None):
                sl = slice(hf * TH, (hf + 1) * TH)
                ybf, ysq = [], []
                for mc in range(DC):
                    yb = ap.tile([P, TH], BF, name=f"yb{mc}", tag=f"h{mc}")
                    nc.scalar.copy(yb[:], ysrc[mc][:])
                    ybf.append(yb)
                    sq = ap.tile([P, TH], BF, name=f"sq{mc}", tag=f"h{4 + mc}")
                    nc.vector.tensor_mul(sq[:], yb[:], yb[:])
                    ysq.append(sq)
                ps_s = psA.tile([P, TH], F32, name="lnS", tag="mm")
                for mc in range(DC):
                    nc.tensor.matmul(ps_s[:], ones[:, 0:P], ybf[mc][:],
                                     start=(mc == 0), stop=(mc == DC - 1))
                ps_q = psA.tile([P, TH], F32, name="lnQ", tag="mm")
                for mc in range(DC):
                    nc.tensor.matmul(ps_q[:], ones[:, 0:P], ysq[mc][:],
                                     start=(mc == 0), stop=(mc == DC - 1))
                stA = ap.tile([P, TH], F32, name="stA", tag="stA")
                stB = ap.tile([P, TH], F32, name="stB", tag="stB")
                stC = ap.tile([P, TH], F32, name="stC", tag="stC")
                stD = ap.tile([P, TH], F32, name="stD", tag="stD")
                nc.vector.tensor_scalar_mul(stA[:], ps_s[:], 1.0 / D)
                nc.vector.tensor_scalar_mul(stB[:], ps_q[:], 1.0 / D)
                nc.vector.tensor_mul(stC[:], stA[:], stA[:])
                nc.vector.tensor_sub(stD[:], stB[:], stC[:])
                nc.scalar.activation(stB[:], stD[:], AF.Sqrt, bias=eps_t[:])
                nc.vector.reciprocal(stC[:], stB[:])
                for mc in range(DC):
                    lt = ap.tile([P, TH], F32, name="lt", tag="lt", bufs=2)
                    nc.vector.tensor_sub(lt[:], ysrc[mc][:], stA[:])
                    lu = ap.tile([P, TH], F32, name="lu", tag="lu", bufs=2)
                    nc.vector.tensor_mul(lu[:], lt[:], stC[:])
                    gcol = ln_t[:, g_off * DC + mc:g_off * DC + mc + 1]
                    bcol = ln_t[:, b_off * DC + mc:b_off * DC + mc + 1]
                    if dstbf is not None:
                        nc.scalar.activation(dstbf[mc][:, sl], lu[:],
                                             AF.Identity, bias=bcol, scale=gcol)
                    if dst8 is not None:
                        g8 = ln_t[:, 4 * DC + mc:4 * DC + mc + 1]
                        b8 = ln_t[:, 5 * DC + mc:5 * DC + mc + 1]
                        o8 = dst8[mc // 2][:, (mc % 2) * S + hf * TH:
                                           (mc % 2) * S + (hf + 1) * TH]
                        nc.scalar.activation(o8, lu[:], AF.Identity,
                                             bias=b8, scale=g8)
                    nc.scalar.activation(dst32[mc][:, sl], lu[:],
                                         AF.Identity, bias=bcol, scale=gcol)

            for hf in range(2):
                sl = slice(hf * TH, (hf + 1) * TH)
                y32h = []
                for mc in range(DC):
                    pt = psA.tile([P, TH], F32, name="pP", tag="mm")
                    for u in range(2):
                        rhs = oT8[u][:].rearrange("p (two m) -> p two m", two=2)
                        nc.tensor.matmul(pt[:], w3o[u][:, :, mc * P:(mc + 1) * P],
                                         rhs[:, :, sl], start=(u == 0),
                                         stop=False, perf_mode=DR)
                    nc.tensor.matmul(pt[:], bo_r[0:1, mc * P:(mc + 1) * P],
                                     ones[0:1, :], start=False, stop=True,
                                     skip_group_check=True)
                    yt = ap.tile([P, TH], F32, name=f"y32_{mc}", tag=f"y32_{mc}")
                    nc.vector.scalar_tensor_tensor(yt[:], pt[:], DS_O,
                                                   x32[mc][:, sl],
                                                   OP.mult, OP.add)
                    y32h.append(yt)
                ln_half(hf, y32h, 0, 1, x132, x1bf)

            # ---------------- phase E: MLP (bf16) ----------------
            for hf in range(2):
                sl = slice(hf * TH, (hf + 1) * TH)
                hts = []
                for fc in range(FC):
                    ph = psA.tile([P, TH], F32, name="pH", tag="mm")
                    for kc in range(DC):
                        nc.tensor.matmul(ph[:], w1[kc][:, fc * P:(fc + 1) * P],
                                         x1bf[kc][:, sl],
                                         start=(kc == 0), stop=(kc == DC - 1))
                    ht = ap.tile([P, TH], BF, name=f"ht{fc}", tag=f"h{fc}")
                    nc.scalar.activation(ht[:], ph[:], AF.Relu,
                                         bias=b1_t[:, fc:fc + 1])
                    hts.append(ht)

                y2h = []
                for mc in range(DC):
                    pt = psA.tile([P, TH], F32, name="pY", tag="mm")
                    for fc in range(FC):
                        nc.tensor.matmul(pt[:], w2[fc][:, mc * P:(mc + 1) * P],
                                         hts[fc][:], start=(fc == 0), stop=False)
                    nc.tensor.matmul(pt[:], b2_r[0:1, mc * P:(mc + 1) * P],
                                     ones[0:1, :], start=False, stop=True,
                                     skip_group_check=True)
                    yt = ap.tile([P, TH], F32, name=f"y32b_{mc}", tag=f"y32_{mc}")
                    nc.vector.tensor_add(yt[:], pt[:], x132[mc][:, sl])
                    y2h.append(yt)
                ln_half(hf, y2h, 2, 3, x32, None,
                        None if last else x8)

        for c in range(DC):
            nc.sync.dma_start(out_d[c * P:(c + 1) * P], x32[c][:])

    nc.compile()
    return nc


_CACHE = {}


def _f8c(a, s):
    a = np.ascontiguousarray(np.asarray(a), dtype=np.float32) * s
    return np.clip(a, -240.0, 240.0).astype(ml_dtypes.float8_e4m3)


def _pairs(w, s):
    # w [K, M] fp32 -> [2, 128, 2*M] fp8: pair u holds K-chunks (2u, 2u+1)
    K, M = w.shape
    r = np.asarray(w, np.float32).reshape(K // P // 2, 2, P, M)
    r = np.transpose(r, (0, 2, 1, 3)).reshape(K // P // 2, P, 2 * M)
    return _f8c(r, s)


def _host_prep(qkv_w, qkv_b, out_w, out_b, ln1_g, ln1_b, mlp_w1, mlp_b1,
               mlp_w2, mlp_b2, ln2_g, ln2_b):
    bfc = lambda a: np.ascontiguousarray(np.asarray(a)).astype(ml_dtypes.bfloat16)
    f32c = lambda a: np.ascontiguousarray(np.asarray(a), dtype=np.float32)
    qkv_w = np.asarray(qkv_w)
    bo_eff = np.asarray(out_b) + np.einsum(
        "ld,ldo->lo", np.asarray(qkv_b)[:, 2 * D:].astype(np.float64),
        np.asarray(out_w).astype(np.float64)).astype(np.float32)
    common = {
        "wqk8": np.stack([_pairs(qkv_w[l, :, :2 * D], S_W) for l in range(L)]),
        "wv8": np.stack([_pairs(qkv_w[l, :, 2 * D:], S_W) for l in range(L)]),
        "wo8": np.stack([_pairs(np.asarray(out_w)[l], S_W) for l in range(L)]),
        "w1": bfc(mlp_w1),
        "w2": bfc(mlp_w2),
        "bq": f32c(np.asarray(qkv_b)[:, :D]),
        "bo": bfc(bo_eff * (S_O * S_W)),
        "b1": f32c(mlp_b1),
        "b2": bfc(mlp_b2),
        "lnp": f32c(np.concatenate(
            [np.stack([ln1_g, ln1_b, ln2_g, ln2_b], axis=1),
             np.stack([np.asarray(ln2_g) * S_X, np.asarray(ln2_b) * S_X],
                      axis=1)], axis=1)),
        "ones": np.ones((P, TH), ml_dtypes.bfloat16),
        "ident": np.eye(P).astype(ml_dtypes.bfloat16),
        "tri": (np.tril(np.ones((P, P), np.float32), -1) * -1e30
                ).astype(ml_dtypes.bfloat16),
    }
    return common


def kernel(x, qkv_w, qkv_b, out_w, out_b, ln1_g, ln1_b, mlp_w1, mlp_b1,
           mlp_w2, mlp_b2, ln2_g, ln2_b):
    if "nc" not in _CACHE:
        _CACHE["nc"] = build()
    nc = _CACHE["nc"]
    common = _host_prep(qkv_w, qkv_b, out_w, out_b, ln1_g, ln1_b, mlp_w1,
                        mlp_b1, mlp_w2, mlp_b2, ln2_g, ln2_b)
    x = np.asarray(x)
    in_maps = []
    for c in range(B):
        xt = np.ascontiguousarray(x[c].T)
        in_maps.append({**common,
                        "x8": _pairs(xt, S_X),
                        "xT32": xt.astype(np.float32)})
    res = run_bass_kernel_spmd(nc, in_maps, core_ids=list(range(B)))
    out = np.stack([np.ascontiguousarray(r["outT"].T) for r in res.results])
    return out.astype(np.float32)
